# revision 12
# baseline (speedup 1.0000x reference)
"""CRaWl GNN forward pass on 8 Trainium2 NeuronCores (Bass/Tile).

Strategy (data-parallel over walks, per sharding hint):
- Each core owns 2048 walks and 2048 nodes.
- conv1/conv2 run as PE matmuls over [cin, (walk, pos)] channel-major tiles,
  contraction tap-by-tap; walk_x taps are quad-packed on host into a
  96-channel shifted tensor so the K dim stays near 128.
- h[walk_nodes] gathers use dma_gather(transpose=True) from an SBUF-resident
  bf16 node table (layers 1-2); layer 0 is host-pre-gathered fp32.
- BatchNorm (training mode) needs global stats: per-channel sum/sumsq are
  accumulated via ACT accum_out during PSUM evacuation, AllReduced, and the
  resulting affine is applied as relu(x + beta/alpha) with alpha folded into
  the next matmul's weights.
- scatter_mean: positions are host-sorted by destination node into
  (node-group, source-stage) cells; per 128-position tile a host-built
  one-hot matrix scatters rows into a [128-node, 128-ch] PSUM accumulator
  via a bf16 matmul.  Counts come from the host (exact).
- Node sums are ReduceScattered (each core gets its 2048-node shard), the
  per-node MLP runs on the shard, and the new node states are AllGathered.
"""
import os
import numpy as np
import ml_dtypes

from concourse import bacc, bass, mybir
import concourse.tile as tile
from concourse.bass_utils import run_bass_kernel_spmd
from concourse.masks import make_identity

BF16 = ml_dtypes.bfloat16
F32 = mybir.dt.float32
F32R = mybir.dt.float32r
BF = mybir.dt.bfloat16
I16 = mybir.dt.int16
AF = mybir.ActivationFunctionType
ALU = mybir.AluOpType

NC_ = 8
P = 128
DBG_NLAYERS = int(os.environ.get('DBG_NLAYERS', '3'))
DBG_STOP = os.environ.get('DBG_STOP', 'FULL')  # A,B,C1,C2,D,FULL
DBG_QROT = os.environ.get('DBG_QROT', '0') == '1'  # >0 queues crash HW

N_NODES = 16384
N_WALKS = 16384
L = 66
KK = 9
LO1 = 58
LO2 = 50
WDIM = 24
FIN = 32
H = 128
CONV = 128
NLAYERS = 3
NGRAPH = 128
OUT = 10
EPS = 1e-5

WPC = N_WALKS // NC_          # 2048 walks per core
NODESPC = N_NODES // NC_      # 2048 nodes per core
NCHUNK = 32                   # chunks of 64 walks
CW = 64                       # walks per chunk
NPOS = WPC * LO2              # 102400 positions per core
NSTAGE = 4
STAGE = NPOS // NSTAGE        # 25600
NGRP = N_NODES // P           # 128 node groups
N1 = N_WALKS * LO1
N2 = N_WALKS * LO2


def _wrap_idx(arr):
    """int16 idx list -> [128, n/16]: idx i at (i%16, i//16), replicated 8x."""
    n = arr.size
    assert n % 16 == 0
    w = arr.reshape(n // 16, 16).T.astype(np.int16)
    return np.tile(w, (8, 1))


def _host_prep(x, walk_x, params, walk_nodes, batch):
    """Build all per-core / shared input arrays."""
    wn_mid = walk_nodes[:, 8:8 + LO2]                    # [W, 50]

    counts = np.bincount(wn_mid.reshape(-1), minlength=N_NODES).astype(np.float64)
    recip_n = (1.0 / np.maximum(counts, 1.0)).astype(np.float32)
    gcounts = np.bincount(batch, minlength=NGRAPH).astype(np.float64)
    recip_g = (1.0 / np.maximum(gcounts, 1.0)).astype(np.float32)

    # ---- scatter sort structure (shared tile grid, per-core data) ----
    key_all, order_all, cnt_all = [], [], []
    for c in range(NC_):
        flat = wn_mid[c * WPC:(c + 1) * WPC].reshape(-1).astype(np.int64)
        j = np.arange(NPOS, dtype=np.int64)
        key = (flat // P) * NSTAGE + (j // STAGE)        # cell id in [0, 512)
        order = np.argsort(key, kind='stable')
        cnt = np.bincount(key, minlength=NGRP * NSTAGE)
        key_all.append(key); order_all.append(order); cnt_all.append(cnt)
    cnt_max = np.max(np.stack(cnt_all), axis=0)          # [512]
    T_cell = np.maximum((cnt_max + P - 1) // P, 1).astype(np.int64)  # tiles/cell
    NT = int(T_cell.sum())

    sidx_cores, oh_cores = [], []
    for c in range(NC_):
        flat = wn_mid[c * WPC:(c + 1) * WPC].reshape(-1).astype(np.int64)
        order, cnt = order_all[c], cnt_all[c]
        starts = np.zeros(NGRP * NSTAGE + 1, np.int64)
        np.cumsum(cnt_all[c], out=starts[1:])
        offs = np.zeros(NT * P, np.int64)                # gather offsets (stage-rel)
        node_loc = np.full(NT * P, -1, np.int64)         # one-hot column (-1 = pad)
        slot = 0
        for cell in range(NGRP * NSTAGE):
            n = int(cnt[cell]); t = int(T_cell[cell])
            seg = order[starts[cell]:starts[cell] + n]
            s = cell % NSTAGE
            offs[slot:slot + n] = seg - s * STAGE
            node_loc[slot:slot + n] = flat[seg] % P
            slot += t * P
        assert offs.min() >= 0 and offs.max() < 32768
        oh = np.zeros((NT * P, P), BF16)
        valid = node_loc >= 0
        oh[np.nonzero(valid)[0], node_loc[valid]] = BF16(1.0)
        sidx_cores.append(_wrap_idx(offs.astype(np.int16)))
        oh_cores.append(oh)                               # [NT*128, 128] bf16

    # ---- h-gather indices (layers 1-2), per core ----
    gidx_cores = [_wrap_idx(walk_nodes[c * WPC:(c + 1) * WPC].reshape(-1)
                            .astype(np.int16)) for c in range(NC_)]

    # ---- conv inputs ----
    in112_cores, wx96_cores, xT_cores, Mgr_cores, recn_cores = [], [], [], [], []
    for c in range(NC_):
        ws = slice(c * WPC, (c + 1) * WPC)
        gat = x[walk_nodes[ws]].transpose(2, 0, 1)        # [32, 2048, 66]
        wx = walk_x[ws].transpose(1, 0, 2)                # [24, 2048, 66]
        in56 = np.concatenate([gat, wx], 0)               # [56, 2048, 66]
        in112 = np.zeros((112, WPC, 67), np.float32)
        in112[0:56, :, 0:66] = in56
        in112[56:112, :, 0:65] = in56[:, :, 1:66]
        in112_cores.append(in112)
        wx96 = np.zeros((96, WPC, 69), np.float32)
        for i in range(4):
            wx96[24 * i:24 * (i + 1), :, 0:66 - i] = wx[:, :, i:66]
        wx96_cores.append(wx96)
        ns = slice(c * NODESPC, (c + 1) * NODESPC)
        xT_cores.append(np.ascontiguousarray(x[ns].T))    # [32, 2048]
        Mgr_cores.append((batch[ns, None] == np.arange(NGRAPH)[None, :])
                         .astype(np.float32))             # [2048, 128]
        rn = recip_n[ns]                                  # [2048]
        recn_cores.append(np.ascontiguousarray(rn.reshape(16, 128).T))  # [128,16]

    # ---- weights ----
    Ls = params['layers']
    w0pack = np.zeros((5, 112, 128), np.float32)
    wc1_0 = np.asarray(Ls[0]['wc1'])                      # [128, 56, 9]
    for m, t in enumerate(range(0, 8, 2)):
        w0pack[m, 0:56] = wc1_0[:, :, t].T
        w0pack[m, 56:112] = wc1_0[:, :, t + 1].T
    w0pack[4, 0:56] = wc1_0[:, :, 8].T

    w1h = np.zeros((2, 9, 128, 128), BF16)
    w1x = np.zeros((2, 3, 96, 128), np.float32)
    for li in (1, 2):
        wc1 = np.asarray(Ls[li]['wc1'])                   # [128, 152, 9]
        for t in range(9):
            w1h[li - 1, t] = wc1[:, 0:128, t].T.astype(BF16)
        for m, t in enumerate((0, 4)):
            for i in range(4):
                w1x[li - 1, m, 24 * i:24 * (i + 1)] = wc1[:, 128 + 0:128 + 24, t + i].T
        w1x[li - 1, 2, 0:24] = wc1[:, 128:152, 8].T

    wc2T = np.stack([np.asarray(Ls[li]['wc2']).transpose(1, 2, 0)
                     .reshape(128, 9 * 128) for li in range(3)])   # [3,128,1152]
    wo1T = np.stack([np.asarray(Ls[li]['wo1']) for li in range(3)])  # [3,128,256]
    wo2T = np.stack([np.asarray(Ls[li]['wo2']) for li in range(3)])  # [3,256,128]
    gb1 = np.stack([np.stack([np.asarray(Ls[li]['g1']),
                              np.asarray(Ls[li]['b1'])], 1) for li in range(3)])
    gb2 = np.stack([np.stack([np.asarray(Ls[li]['g2']),
                              np.asarray(Ls[li]['b2'])], 1) for li in range(3)])
    gbo = np.stack([np.stack([np.asarray(Ls[li]['go']),
                              np.asarray(Ls[li]['bo'])], 1) for li in range(3)])  # [3,256,2]
    gbn = np.stack([np.asarray(params['node_g']), np.asarray(params['node_b'])], 1)
    ow2 = np.zeros((128, 16), np.float32)
    ow2[:, 0:10] = np.asarray(params['out_w2'])
    ob2 = np.zeros((16, 1), np.float32)
    ob2[0:10, 0] = np.asarray(params['out_b2'])

    shared = dict(
        w0pack=w0pack, w1h=np.asarray(w1h), w1x=w1x, wc2T=wc2T,
        wo1T=wo1T, wo2T=wo2T, gb1=gb1, gb2=gb2,
        gbo=gbo, gbn=gbn,
        rescale=np.asarray(params['rescale']),            # [32, 128]
        ow1=np.asarray(params['out_w1']),                 # [128, 128]
        ob1=np.asarray(params['out_b1']).reshape(128, 1),
        ow2=ow2, ob2=ob2,
        recip_g=recip_g.reshape(128, 1),
        iota_c1=_wrap_idx(np.arange(3200, dtype=np.int16)),
    )
    per_core = []
    for c in range(NC_):
        per_core.append(dict(
            in112=in112_cores[c], wx96=wx96_cores[c], xT=xT_cores[c],
            gidx=gidx_cores[c], sidx=sidx_cores[c], onehot=oh_cores[c],
            Mgr=Mgr_cores[c], recip_n=recn_cores[c],
        ))
    return shared, per_core, T_cell, NT


def _build(T_cell, NT):
    """Build the 8-core SPMD Bass program."""
    nc = bacc.Bacc("TRN2", target_bir_lowering=False, debug=False,
                   num_devices=NC_, num_swdge_queues=4)
    RG = [list(range(NC_))]

    # ---------------- I/O ----------------
    din112 = nc.dram_tensor("in112", [112, WPC, 67], F32, kind="ExternalInput")
    dwx96 = nc.dram_tensor("wx96", [96, WPC, 69], F32, kind="ExternalInput")
    dxT = nc.dram_tensor("xT", [32, NODESPC], F32, kind="ExternalInput")
    dgidx = nc.dram_tensor("gidx", [128, WPC * L // 16], I16, kind="ExternalInput")
    dsidx = nc.dram_tensor("sidx", [128, NT * 8], I16, kind="ExternalInput")
    doh = nc.dram_tensor("onehot", [NT * P, P], mybir.dt.bfloat16,
                         kind="ExternalInput")
    dMgr = nc.dram_tensor("Mgr", [NODESPC, NGRAPH], F32, kind="ExternalInput")
    drecn = nc.dram_tensor("recip_n", [128, 16], F32, kind="ExternalInput")
    drecg = nc.dram_tensor("recip_g", [128, 1], F32, kind="ExternalInput")
    dw0 = nc.dram_tensor("w0pack", [5, 112, 128], F32, kind="ExternalInput")
    dw1h = nc.dram_tensor("w1h", [2, 9, 128, 128], mybir.dt.bfloat16,
                          kind="ExternalInput")
    dw1x = nc.dram_tensor("w1x", [2, 3, 96, 128], F32, kind="ExternalInput")
    dwc2 = nc.dram_tensor("wc2T", [3, 128, 1152], F32, kind="ExternalInput")
    dwo1 = nc.dram_tensor("wo1T", [3, 128, 256], F32, kind="ExternalInput")
    dwo2 = nc.dram_tensor("wo2T", [3, 256, 128], F32, kind="ExternalInput")
    dgb1 = nc.dram_tensor("gb1", [3, 128, 2], F32, kind="ExternalInput")
    dgb2 = nc.dram_tensor("gb2", [3, 128, 2], F32, kind="ExternalInput")
    dgbo = nc.dram_tensor("gbo", [3, 256, 2], F32, kind="ExternalInput")
    dgbn = nc.dram_tensor("gbn", [128, 2], F32, kind="ExternalInput")
    dresc = nc.dram_tensor("rescale", [32, 128], F32, kind="ExternalInput")
    dow1 = nc.dram_tensor("ow1", [128, 128], F32, kind="ExternalInput")
    dob1 = nc.dram_tensor("ob1", [128, 1], F32, kind="ExternalInput")
    dow2 = nc.dram_tensor("ow2", [128, 16], F32, kind="ExternalInput")
    dob2 = nc.dram_tensor("ob2", [16, 1], F32, kind="ExternalInput")
    diota = nc.dram_tensor("iota_c1", [128, 200], I16, kind="ExternalInput")
    dy = nc.dram_tensor("y", [16, NGRAPH], F32, kind="ExternalOutput")

    # internal DRAM
    c1st = nc.dram_tensor("c1store", [128, WPC * LO1], mybir.dt.bfloat16)
    c2st = nc.dram_tensor("c2store", [128, NPOS], mybir.dt.bfloat16)
    posmaj = nc.dram_tensor("posmaj", [NPOS, 128], mybir.dt.bfloat16)

    cum_T = np.zeros(NGRP * NSTAGE + 1, np.int64)
    np.cumsum(T_cell, out=cum_T[1:])
    Tg_tot = [int(T_cell[g * NSTAGE:(g + 1) * NSTAGE].sum()) for g in range(NGRP)]
    Tg_max = max(Tg_tot)
    Ts_max = int(T_cell.max())

    with tile.TileContext(nc) as tc:
     with (
        tc.tile_pool(name="persist", bufs=1) as pers,
        tc.tile_pool(name="dram", bufs=1, space="DRAM") as dpool,
     ):
        # ---------------- persistent SBUF ----------------
        h_sb = pers.tile([128, 128, 128], BF)        # node table (n%128, n//128, f)
        gidx_sb = pers.tile([128, WPC * L // 16], I16)
        sidx_sb = pers.tile([128, NT * 8], I16)
        iota_sb = pers.tile([128, 200], I16)
        w0_sb = pers.tile([112, 5 * 128], F32)
        w1h_sb = pers.tile([128, 2 * 9 * 128], BF)
        w1x_sb = pers.tile([96, 2 * 3 * 128], F32)
        wc2_sb = pers.tile([128, 1152], F32)         # per-layer raw
        w2f_sb = pers.tile([128, 1152], F32)         # folded
        wo1_sb = pers.tile([128, 256], F32)
        wo1f_sb = pers.tile([128, 256], F32)
        wo2_sb = pers.tile([128, 2 * 128], F32)      # [2 halves][128,128] stacked free
        wo2f_sb = pers.tile([128, 2 * 128], F32)
        resc_sb = pers.tile([32, 128], F32)
        xT_sb = pers.tile([32, NODESPC], F32)
        recn_sb = pers.tile([128, 16], F32)
        recg_sb = pers.tile([128, 1], F32)
        gb1_sb = pers.tile([128, 2 * 3], F32)
        gb2_sb = pers.tile([128, 2 * 3], F32)
        gbo_sb = pers.tile([128, 2 * 2 * 3], F32)    # per half: [128, 2] x2 x3
        gbn_sb = pers.tile([128, 2], F32)
        ow1_sb = pers.tile([128, 128], F32)
        ob1_sb = pers.tile([128, 1], F32)
        ow2_sb = pers.tile([128, 16], F32)
        ob2_sb = pers.tile([16, 1], F32)
        ident = pers.tile([128, 128], F32)
        hlocA = pers.tile([128, NODESPC], F32)
        hlocB = pers.tile([128, NODESPC], F32)
        stat_s = pers.tile([128, 64], F32)
        stat_q = pers.tile([128, 64], F32)
        bn_sc = pers.tile([128, 12], F32)            # scratch for bn math
        sqs = pers.tile([128, 4, 512], F32)          # square scratch

        eps_ap = pers.tile([128, 1], F32)
        nc.vector.memset(eps_ap[:], EPS)
        make_identity(nc, ident[:])
        nc.sync.dma_start(gidx_sb[:], dgidx[:])
        nc.sync.dma_start(sidx_sb[:], dsidx[:])
        nc.sync.dma_start(iota_sb[:], diota[:])
        nc.sync.dma_start(w0_sb[:].rearrange("k (m c) -> k m c", m=5).bitcast(F32R),
                          dw0[:].transpose([1, 0, 2]).bitcast(F32R))
        nc.sync.dma_start(w1h_sb[:].rearrange("k (l t c) -> k l t c", l=2, t=9), dw1h[:].transpose([2, 0, 1, 3]))
        nc.sync.dma_start(w1x_sb[:].rearrange("k (l m c) -> k l m c", l=2, m=3).bitcast(F32R),
                          dw1x[:].transpose([2, 0, 1, 3]).bitcast(F32R))
        nc.sync.dma_start(resc_sb[:].bitcast(F32R), dresc[:].bitcast(F32R))
        nc.sync.dma_start(xT_sb[:].bitcast(F32R), dxT[:].bitcast(F32R))
        nc.sync.dma_start(recn_sb[:], drecn[:])
        nc.sync.dma_start(recg_sb[:], drecg[:])
        nc.sync.dma_start(gb1_sb[:].rearrange("k (l two) -> k l two", l=3), dgb1[:].transpose([1, 0, 2]))
        nc.sync.dma_start(gb2_sb[:].rearrange("k (l two) -> k l two", l=3), dgb2[:].transpose([1, 0, 2]))
        nc.sync.dma_start(gbo_sb[:].rearrange("k (l h two) -> k l h two", l=3, h=2),
                          dgbo[:].rearrange("l (h k) two -> l h k two", k=128).transpose([2, 0, 1, 3]))
        nc.sync.dma_start(gbn_sb[:], dgbn[:])
        nc.sync.dma_start(ow1_sb[:], dow1[:])
        nc.sync.dma_start(ob1_sb[:], dob1[:])
        nc.sync.dma_start(ow2_sb[:], dow2[:])
        nc.sync.dma_start(ob2_sb[:], dob2[:])

        # layer-0 residual: hlocA = rescale.T @ xT
        with tc.tile_pool(name="ps_init", bufs=2, space="PSUM") as pp0:
            for nb in range(4):
                psr = pp0.tile([128, 512], F32, space="PSUM")
                nc.tensor.matmul(psr[:], resc_sb[:].bitcast(F32R),
                                 xT_sb[:, nb * 512:(nb + 1) * 512].bitcast(F32R),
                                 start=True, stop=True)
                nc.vector.tensor_copy(hlocA[:, nb * 512:(nb + 1) * 512], psr[:])

        def bn_fold(ar_sb, npos_total, gb_ap, a_out, bia_out):  # noqa
            """ar_sb [128, 2] (sum, sumsq) -> a_out [128,1], bia_out [128,1].

            a = g / sqrt(var + eps); bia = b/a - m  (apply as a*relu(x+bia))."""
            m_ = bn_sc[:, 0:1]; q_ = bn_sc[:, 1:2]; v_ = bn_sc[:, 2:3]
            sd = bn_sc[:, 3:4]; rs_ = bn_sc[:, 4:5]; ra = bn_sc[:, 5:6]
            t_ = bn_sc[:, 6:7]
            nc.vector.tensor_scalar(m_, ar_sb[:, 0:1], 1.0 / npos_total, None,
                                    op0=ALU.mult)
            nc.vector.tensor_scalar(q_, ar_sb[:, 1:2], 1.0 / npos_total, None,
                                    op0=ALU.mult)
            nc.vector.tensor_tensor(t_, m_, m_, op=ALU.mult)
            nc.vector.tensor_tensor(v_, q_, t_, op=ALU.subtract)
            nc.scalar.activation(sd, v_, AF.Sqrt, bias=eps_ap[:, 0:1])
            nc.vector.reciprocal(rs_, sd)
            nc.vector.tensor_tensor(a_out, gb_ap[:, 0:1], rs_, op=ALU.mult)
            nc.vector.reciprocal(ra, a_out)
            nc.vector.tensor_tensor(t_, gb_ap[:, 1:2], ra, op=ALU.mult)
            nc.vector.tensor_tensor(bia_out, t_, m_, op=ALU.subtract)

        _last_li = min(NLAYERS, DBG_NLAYERS) - 1

        def _stop_after(phase, _li=None):
            order = ['A', 'B', 'C1', 'C2', 'D', 'FULL']
            return (_li == _last_li
                    and order.index(DBG_STOP) <= order.index(phase))

        for li in range(min(NLAYERS, DBG_NLAYERS)):
            lw = li - 1  # index into w1h/w1x
            a1 = pers.tile([128, 1], F32, tag=f"a1_{li}")
            bia1 = pers.tile([128, 1], F32, tag=f"bia1_{li}")
            a2 = pers.tile([128, 1], F32, tag=f"a2_{li}")
            bia2 = pers.tile([128, 1], F32, tag=f"bia2_{li}")

            # ---------------- phase A: conv1 + stats ----------------
            with (
                tc.tile_pool(name="A_in", bufs=2) as pin,
                tc.tile_pool(name="A_out", bufs=2) as pout,
                tc.tile_pool(name="A_ps", bufs=2, space="PSUM") as pps,
            ):
                for k in range(NCHUNK):
                    if li == 0:
                        cin = pin.tile([112, CW, 67], F32, tag="conv_in")
                        nc.sync.dma_start(
                            cin[:].bitcast(F32R),
                            din112[:, k * CW:(k + 1) * CW, :].bitcast(F32R))
                    else:
                        ghat = pin.tile([128, 1, CW * L], BF, tag="ghat")
                        # transpose dma_gather caps at ~896 idx/call (64-desc
                        # single-packet ceiling); chunk 4224 -> 6 x 704 and
                        # rotate SWDGE queues to spread Q7 prep cost.
                        icol0 = k * (CW * L // 16)
                        gsplit = (0, 768, 1536, 2304, 3072, 3840, 4224)
                        for gg in range(6):
                            g0, g1 = gsplit[gg], gsplit[gg + 1]
                            nc.gpsimd.dma_gather(
                                ghat[:, :, g0:g1],
                                h_sb[:].rearrange("p r f -> p (r f)"),
                                gidx_sb[:, icol0 + g0 // 16:icol0 + g1 // 16],
                                num_idxs=g1 - g0, num_idxs_reg=g1 - g0,
                                elem_size=128, transpose=True,
                                sbuf_tokens_per_rank=128,
                                sbuf_free_dim_per_rank=256,
                                queue_num=(gg % 4) if DBG_QROT else 0)
                        wxc = pin.tile([96, CW, 69], F32, tag="conv_in")
                        nc.sync.dma_start(
                            wxc[:].bitcast(F32R),
                            dwx96[:, k * CW:(k + 1) * CW, :].bitcast(F32R))
                    for half in range(2):
                        ps = pps.tile([128, 4, 512], F32, space="PSUM")
                        for s4 in range(4):
                            st = half * 4 + s4
                            out_ap = ps[:, s4, 0:464]
                            first = True
                            if li == 0:
                                for m, toff in ((0, 0), (1, 2), (2, 4), (3, 6), (4, 8)):
                                    nc.tensor.matmul(
                                        out_ap,
                                        w0_sb[:, m * 128:(m + 1) * 128].bitcast(F32R),
                                        cin[:, st * 8:st * 8 + 8, toff:toff + LO1]
                                        .bitcast(F32R),
                                        start=first, stop=(m == 4),
                                        skip_group_check=True)
                                    first = False
                            else:
                                gv = ghat[:].rearrange("p one (w q) -> p (one w) q", w=CW)
                                for t in range(9):
                                    nc.tensor.matmul(
                                        out_ap,
                                        w1h_sb[:, (lw * 9 + t) * 128:(lw * 9 + t + 1) * 128],
                                        gv[:, st * 8:st * 8 + 8, t:t + LO1],
                                        start=first, stop=False,
                                        skip_group_check=True)
                                    first = False
                                for m, toff in ((0, 0), (1, 4), (2, 8)):
                                    nc.tensor.matmul(
                                        out_ap,
                                        w1x_sb[:, (lw * 3 + m) * 128:(lw * 3 + m) * 128 + 128]
                                        .bitcast(F32R),
                                        wxc[:, st * 8:st * 8 + 8, toff:toff + LO1]
                                        .bitcast(F32R),
                                        start=False, stop=(m == 2),
                                        skip_group_check=True)
                        c1t = pout.tile([128, 4, 464], BF, tag="c1t")
                        col = 2 * k + half
                        nc.scalar.activation(c1t[:], ps[:, :, 0:464], AF.Copy,
                                             accum_out=stat_s[:, col:col + 1])
                        nc.scalar.activation(sqs[:, :, 0:464],
                                             ps[:, :, 0:464], AF.Square,
                                             accum_out=stat_q[:, col:col + 1])
                        nc.sync.dma_start(
                            c1st[:, (k * 2 + half) * 1856:(k * 2 + half + 1) * 1856]
                            .rearrange("p (s q) -> p s q", s=4),
                            c1t[:])

                # bn1 stats allreduce
                s1 = bn_sc[:, 8:9]; q1 = bn_sc[:, 9:10]
                nc.vector.tensor_reduce(s1, stat_s[:], axis=mybir.AxisListType.X,
                                        op=ALU.add)
                nc.vector.tensor_reduce(q1, stat_q[:], axis=mybir.AxisListType.X,
                                        op=ALU.add)
                arin = dpool.tile([128, 2], F32, tag=f"bn1in_{li}")
                arout = dpool.tile([128, 2], F32, tag=f"bn1out_{li}")
                st2 = pout.tile([128, 2], F32, tag="st2")
                nc.vector.tensor_copy(st2[:, 0:1], s1)
                nc.vector.tensor_copy(st2[:, 1:2], q1)
                nc.sync.dma_start(arin[:], st2[:])
                nc.gpsimd.collective_compute(
                    "AllReduce", ALU.add, replica_groups=RG,
                    ins=[arin[:].opt()], outs=[arout[:].opt()])
                ar1 = pout.tile([128, 2], F32, tag="ar1")
                nc.sync.dma_start(ar1[:], arout[:])
                bn_fold(ar1, N1, gb1_sb[:, 2 * li:2 * li + 2], a1[:], bia1[:])
                # fold a1 into wc2
                nc.sync.dma_start(wc2_sb[:], dwc2[li])
                nc.vector.tensor_scalar(w2f_sb[:].bitcast(F32R), wc2_sb[:],
                                        a1[:, 0:1], None, op0=ALU.mult)

            if _stop_after('A', li):
                break
            # ---------------- phase B: conv2 + stats ----------------
            with (
                tc.tile_pool(name="B_in", bufs=2) as pin,
                tc.tile_pool(name="B_act", bufs=2) as pact,
                tc.tile_pool(name="B_out", bufs=2) as pout,
                tc.tile_pool(name="B_ps", bufs=2, space="PSUM") as pps,
            ):
                for k in range(NCHUNK):
                    c1in = pin.tile([128, CW, LO1], BF, tag="c1in")
                    nc.sync.dma_start(
                        c1in[:],
                        c1st[:, k * CW * LO1:(k + 1) * CW * LO1]
                        .rearrange("p (w q) -> p w q", w=CW))
                    act1 = pact.tile([128, CW, LO1], F32, tag="act1")
                    nc.scalar.activation(act1[:].bitcast(F32R), c1in[:],
                                         AF.Relu, bias=bia1[:, 0:1])
                    for half in range(2):
                        ps = pps.tile([128, 4, 512], F32, space="PSUM")
                        for s4 in range(4):
                            st = half * 4 + s4
                            out_ap = ps[:, s4, 0:400]
                            for t in range(9):
                                nc.tensor.matmul(
                                    out_ap,
                                    w2f_sb[:, t * 128:(t + 1) * 128].bitcast(F32R),
                                    act1[:, st * 8:st * 8 + 8, t:t + LO2]
                                    .bitcast(F32R),
                                    start=(t == 0), stop=(t == 8),
                                    skip_group_check=True)
                        c2t = pout.tile([128, 4, 400], BF, tag="c2t")
                        col = 2 * k + half + 0
                        nc.scalar.activation(c2t[:], ps[:, :, 0:400], AF.Copy,
                                             accum_out=stat_s[:, col:col + 1])
                        nc.scalar.activation(sqs[:, :, 0:400],
                                             ps[:, :, 0:400], AF.Square,
                                             accum_out=stat_q[:, col:col + 1])
                        nc.sync.dma_start(
                            c2st[:, (k * 2 + half) * 1600:(k * 2 + half + 1) * 1600]
                            .rearrange("p (s q) -> p s q", s=4),
                            c2t[:])
                s2 = bn_sc[:, 8:9]; q2 = bn_sc[:, 9:10]
                nc.vector.tensor_reduce(s2, stat_s[:], axis=mybir.AxisListType.X,
                                        op=ALU.add)
                nc.vector.tensor_reduce(q2, stat_q[:], axis=mybir.AxisListType.X,
                                        op=ALU.add)
                arin = dpool.tile([128, 2], F32, tag=f"bn2in_{li}")
                arout = dpool.tile([128, 2], F32, tag=f"bn2out_{li}")
                st2 = pout.tile([128, 2], F32, tag="st2b")
                nc.vector.tensor_copy(st2[:, 0:1], s2)
                nc.vector.tensor_copy(st2[:, 1:2], q2)
                nc.sync.dma_start(arin[:], st2[:])
                nc.gpsimd.collective_compute(
                    "AllReduce", ALU.add, replica_groups=RG,
                    ins=[arin[:].opt()], outs=[arout[:].opt()])
                ar2 = pout.tile([128, 2], F32, tag="ar2")
                nc.sync.dma_start(ar2[:], arout[:])
                bn_fold(ar2, N2, gb2_sb[:, 2 * li:2 * li + 2], a2[:], bia2[:])
                nc.sync.dma_start(wo1_sb[:], dwo1[li])
                nc.vector.tensor_scalar(wo1f_sb[:].bitcast(F32R), wo1_sb[:],
                                        a2[:, 0:1], None, op0=ALU.mult)

            if _stop_after('B', li):
                break
            # ---------------- phase C1: bn2relu + transpose to posmaj --------
            with (
                tc.tile_pool(name="C1_in", bufs=2) as pin,
                tc.tile_pool(name="C1_t", bufs=2) as ptr,
            ):
                for k in range(NCHUNK):
                    c2in = pin.tile([128, 3200], BF, tag="c2in")
                    nc.sync.dma_start(c2in[:],
                                      c2st[:, k * 3200:(k + 1) * 3200])
                    fbn = pin.tile([128, 3200], BF, tag="fbn")
                    nc.scalar.activation(fbn[:], c2in[:], AF.Relu,
                                         bias=bia2[:, 0:1])
                    ptile = ptr.tile([128, 1, 3200], BF, tag="ptile")
                    psplit = (0, 768, 1536, 2304, 3200)
                    for gg in range(4):
                        g0, g1 = psplit[gg], psplit[gg + 1]
                        nc.gpsimd.dma_gather(
                            ptile[:, :, g0:g1],
                            fbn[:], iota_sb[:, g0 // 16:g1 // 16],
                            num_idxs=g1 - g0, num_idxs_reg=g1 - g0,
                            elem_size=128, transpose=True,
                            sbuf_tokens_per_rank=128,
                            sbuf_free_dim_per_rank=256,
                            queue_num=(gg % 4) if DBG_QROT else 0)
                    nc.sync.dma_start(
                        posmaj[:].rearrange("(kk jb p) c -> kk jb p c",
                                            kk=NCHUNK, jb=25)[k].transpose([1, 0, 2]),
                        ptile[:].rearrange("p one (jb c) -> p (one jb) c", jb=25))

            if _stop_after('C1', li):
                break
            # ---------------- phase C2: sorted one-hot scatter ----------------
            sums = dpool.tile([N_NODES, 128], F32, tag=f"sums_{li}")
            rsout = dpool.tile([NODESPC, 128], F32, tag=f"rsout_{li}")
            with (
                tc.tile_pool(name="C2_f", bufs=3) as pf,
                tc.tile_pool(name="C2_oh", bufs=2) as poh,
                tc.tile_pool(name="C2_o", bufs=2) as po,
                tc.tile_pool(name="C2_ps", bufs=4, space="PSUM") as pps,
            ):
                for g in range(NGRP):
                    ohg = poh.tile([128, Tg_max, 128], BF, tag="ohg")
                    base_t = int(cum_T[g * NSTAGE])
                    tg = Tg_tot[g]
                    nc.sync.dma_start(
                        ohg[:, 0:tg, :],
                        doh[:].rearrange("(n p) c -> n p c", p=128)
                        [base_t:base_t + tg].transpose([1, 0, 2]))
                    psS = pps.tile([128, 512], F32, space="PSUM")
                    mm_i = 0
                    for s in range(NSTAGE):
                        cell = g * NSTAGE + s
                        Tc = int(T_cell[cell])
                        ct = int(cum_T[cell])
                        ftile = pf.tile([128, Ts_max, 128], BF, tag="ftile")
                        nc.gpsimd.dma_gather(
                            ftile[:, 0:Tc, :],
                            posmaj[s * STAGE:(s + 1) * STAGE, :],
                            sidx_sb[:, ct * 8:(ct + Tc) * 8],
                            num_idxs=Tc * 128, num_idxs_reg=Tc * 128,
                            elem_size=128, transpose=False)
                        for t in range(Tc):
                            nc.tensor.matmul(
                                psS[:, 0:128],
                                ohg[:, (ct - base_t) + t, :],
                                ftile[:, t, :],
                                start=(mm_i == 0), stop=(mm_i == tg - 1),
                                skip_group_check=True)
                            mm_i += 1
                    ssum = po.tile([128, 128], F32, tag="ssum")
                    nc.vector.tensor_copy(ssum[:], psS[:, 0:128])
                    nc.sync.dma_start(sums[g * 128:(g + 1) * 128, :], ssum[:])
                nc.gpsimd.collective_compute(
                    "ReduceScatter", ALU.add, replica_groups=RG,
                    ins=[sums[:].opt()], outs=[rsout[:].opt()])

            if _stop_after('C2', li):
                break
            # ---------------- phase D: node MLP on shard ----------------
            hloc = hlocA if li % 2 == 0 else hlocB
            hnew = hlocB if li % 2 == 0 else hlocA
            hsh = dpool.tile([NODESPC, 128], mybir.dt.bfloat16, tag=f"hsh_{li}")
            hfull = dpool.tile([N_NODES, 128], mybir.dt.bfloat16, tag=f"hfull_{li}")
            with (
                tc.tile_pool(name="D_sb", bufs=1) as psb,
                tc.tile_pool(name="D_ps", bufs=2, space="PSUM") as pps,
            ):
                sums_nm = psb.tile([128, 16, 128], F32)
                nc.sync.dma_start(
                    sums_nm[:], rsout[:].rearrange("(j p) c -> j p c", p=128).transpose([1, 0, 2]))
                hmT = psb.tile([128, NODESPC], F32)
                for j in range(16):
                    nc.vector.tensor_scalar(sums_nm[:, j, :], sums_nm[:, j, :],
                                            recn_sb[:, j:j + 1], None, op0=ALU.mult)
                    psT = pps.tile([128, 512], F32, space="PSUM", tag="psT")
                    nc.tensor.transpose(psT[:, 0:128], sums_nm[:, j, :], ident[:])
                    nc.vector.tensor_copy(hmT[:, j * 128:(j + 1) * 128]
                                          .bitcast(F32R), psT[:, 0:128])
                z1 = psb.tile([128, 2, NODESPC], F32)
                zscr = psb.tile([128, 512], F32)
                for mh in range(2):
                    for nb in range(4):
                        psZ = pps.tile([128, 512], F32, space="PSUM", tag="psZ")
                        nc.tensor.matmul(
                            psZ[:], wo1f_sb[:, mh * 128:(mh + 1) * 128].bitcast(F32R),
                            hmT[:, nb * 512:(nb + 1) * 512].bitcast(F32R),
                            start=True, stop=True, skip_group_check=True)
                        col = mh * 4 + nb
                        nc.scalar.activation(z1[:, mh, nb * 512:(nb + 1) * 512],
                                             psZ[:], AF.Copy,
                                             accum_out=stat_s[:, col:col + 1])
                        nc.scalar.activation(zscr[:], psZ[:], AF.Square,
                                             accum_out=stat_q[:, col:col + 1])
                # z stats AR: [128, 4] = (sum_h0, sum_h1, sq_h0, sq_h1)
                zst = psb.tile([128, 4], F32)
                nc.vector.tensor_reduce(zst[:, 0:1], stat_s[:, 0:4],
                                        axis=mybir.AxisListType.X, op=ALU.add)
                nc.vector.tensor_reduce(zst[:, 1:2], stat_s[:, 4:8],
                                        axis=mybir.AxisListType.X, op=ALU.add)
                nc.vector.tensor_reduce(zst[:, 2:3], stat_q[:, 0:4],
                                        axis=mybir.AxisListType.X, op=ALU.add)
                nc.vector.tensor_reduce(zst[:, 3:4], stat_q[:, 4:8],
                                        axis=mybir.AxisListType.X, op=ALU.add)
                arin = dpool.tile([128, 4], F32, tag=f"zin_{li}")
                arout = dpool.tile([128, 4], F32, tag=f"zout_{li}")
                nc.sync.dma_start(arin[:], zst[:])
                nc.gpsimd.collective_compute(
                    "AllReduce", ALU.add, replica_groups=RG,
                    ins=[arin[:].opt()], outs=[arout[:].opt()])
                arz = psb.tile([128, 4], F32)
                nc.sync.dma_start(arz[:], arout[:])
                nc.sync.dma_start(wo2_sb[:].rearrange("k (h c) -> k h c", h=2),
                                  dwo2[li].rearrange("(h k) c -> h k c", k=128).transpose([1, 0, 2]))
                zr = psb.tile([128, 2, NODESPC], F32)
                for mh in range(2):
                    ao = bn_sc[:, 10:11]; biao = bn_sc[:, 11:12]
                    arh = psb.tile([128, 2], F32, tag=f"arh")
                    nc.vector.tensor_copy(arh[:, 0:1], arz[:, mh:mh + 1])
                    nc.vector.tensor_copy(arh[:, 1:2], arz[:, 2 + mh:3 + mh])
                    bn_fold(arh, N_NODES,
                            gbo_sb[:, li * 4 + mh * 2: li * 4 + mh * 2 + 2],
                            ao, biao)
                    nc.vector.tensor_scalar(
                        wo2f_sb[:, mh * 128:(mh + 1) * 128].bitcast(F32R),
                        wo2_sb[:, mh * 128:(mh + 1) * 128],
                        ao, None, op0=ALU.mult)
                    nc.scalar.activation(zr[:, mh, :].bitcast(F32R),
                                         z1[:, mh, :], AF.Relu, bias=biao)
                for nb in range(4):
                    psH = pps.tile([128, 512], F32, space="PSUM", tag="psH")
                    for mh in range(2):
                        nc.tensor.matmul(
                            psH[:], wo2f_sb[:, mh * 128:(mh + 1) * 128].bitcast(F32R),
                            zr[:, mh, nb * 512:(nb + 1) * 512].bitcast(F32R),
                            start=(mh == 0), stop=(mh == 1),
                            skip_group_check=True)
                    nc.vector.tensor_tensor(hnew[:, nb * 512:(nb + 1) * 512],
                                            psH[:], hloc[:, nb * 512:(nb + 1) * 512],
                                            op=ALU.add)
                if li < NLAYERS - 1:
                    hnode = psb.tile([128, 16, 128], BF)
                    for j in range(16):
                        psT = pps.tile([128, 512], F32, space="PSUM", tag="psT")
                        nc.tensor.transpose(psT[:, 0:128],
                                            hnew[:, j * 128:(j + 1) * 128], ident[:])
                        nc.vector.tensor_copy(hnode[:, j, :], psT[:, 0:128])
                    nc.sync.dma_start(
                        hsh[:].rearrange("(j p) c -> j p c", p=128).transpose([1, 0, 2]), hnode[:])
                    nc.gpsimd.collective_compute(
                        "AllGather", ALU.bypass, replica_groups=RG,
                        ins=[hsh[:].opt()], outs=[hfull[:].opt()])
                    nc.sync.dma_start(
                        h_sb[:], hfull[:].rearrange("(r p) f -> r p f", p=128).transpose([1, 0, 2]))

        # ---------------- final head ----------------
        if DBG_STOP != 'FULL' or DBG_NLAYERS < 3:
            dummy = pers.tile([16, 128], F32)
            nc.vector.memset(dummy[:], 1.0)
            nc.sync.dma_start(dy[:], dummy[:])
            do_head = False
        else:
            do_head = True
        hfin = hlocB if NLAYERS % 2 == 1 else hlocA
        if do_head:
         with (
            tc.tile_pool(name="F_sb", bufs=1) as psb,
            tc.tile_pool(name="F_ps", bufs=1, space="PSUM") as pps,
        ):
            # final node bn stats
            scr = psb.tile([128, NODESPC], F32)
            fst = psb.tile([128, 2], F32)
            nc.scalar.activation(scr[:], hfin[:], AF.Copy,
                                 accum_out=fst[:, 0:1])
            nc.scalar.activation(scr[:], hfin[:], AF.Square,
                                 accum_out=fst[:, 1:2])
            arin = dpool.tile([128, 2], F32, tag="fin")
            arout = dpool.tile([128, 2], F32, tag="fout")
            nc.sync.dma_start(arin[:], fst[:])
            nc.gpsimd.collective_compute(
                "AllReduce", ALU.add, replica_groups=RG,
                ins=[arin[:].opt()], outs=[arout[:].opt()])
            arf = psb.tile([128, 2], F32)
            nc.sync.dma_start(arf[:], arout[:])
            af = bn_sc[:, 10:11]; biaf = bn_sc[:, 11:12]
            bn_fold(arf, N_NODES, gbn_sb[:], af, biaf)
            hn = psb.tile([128, NODESPC], F32)
            nc.scalar.activation(hn[:], hfin[:], AF.Relu, bias=biaf)
            # node-major + pooling matmuls
            Mgr_sb = psb.tile([128, 16, 128], F32)
            nc.sync.dma_start(Mgr_sb[:],
                              dMgr[:].rearrange("(j p) g -> j p g", p=128).transpose([1, 0, 2]))
            hn_nm = psb.tile([128, 16, 128], F32)
            for j in range(16):
                psT = pps.tile([128, 512], F32, space="PSUM", tag="psT2")
                nc.tensor.transpose(psT[:, 0:128],
                                    hn[:, j * 128:(j + 1) * 128], ident[:])
                nc.vector.tensor_copy(hn_nm[:, j, :], psT[:, 0:128])
            psP = pps.tile([128, 512], F32, space="PSUM", tag="psP")
            for j in range(16):
                nc.tensor.matmul(psP[:, 0:128], Mgr_sb[:, j, :], hn_nm[:, j, :],
                                 start=(j == 0), stop=(j == 15),
                                 skip_group_check=True)
            pooled = psb.tile([128, 128], F32)
            nc.vector.tensor_copy(pooled[:], psP[:, 0:128])
            arpin = dpool.tile([128, 128], F32, tag="pin")
            arpout = dpool.tile([128, 128], F32, tag="pout")
            nc.sync.dma_start(arpin[:], pooled[:])
            nc.gpsimd.collective_compute(
                "AllReduce", ALU.add, replica_groups=RG,
                ins=[arpin[:].opt()], outs=[arpout[:].opt()])
            xg = psb.tile([128, 128], F32)
            nc.sync.dma_start(xg[:], arpout[:])
            nc.vector.tensor_scalar(xg[:], xg[:], recg_sb[:, 0:1], None,
                                    op0=ALU.mult)
            psX = pps.tile([128, 512], F32, space="PSUM", tag="psX")
            nc.tensor.transpose(psX[:, 0:128], xg[:], ident[:])
            xgT = psb.tile([128, 128], F32)
            nc.vector.tensor_scalar(xgT[:], psX[:, 0:128], af, None, op0=ALU.mult)
            psY = pps.tile([128, 512], F32, space="PSUM", tag="psY")
            nc.tensor.matmul(psY[:, 0:128], ow1_sb[:], xgT[:],
                             start=True, stop=True, skip_group_check=True)
            y1 = psb.tile([128, 128], F32)
            nc.scalar.activation(y1[:], psY[:, 0:128], AF.Relu, bias=ob1_sb[:, 0:1])
            psY2 = pps.tile([16, 512], F32, space="PSUM", tag="psY2")
            nc.tensor.matmul(psY2[:, 0:128], ow2_sb[:], y1[:],
                             start=True, stop=True, skip_group_check=True)
            y2 = psb.tile([16, 128], F32)
            nc.scalar.activation(y2[:], psY2[:, 0:128], AF.Identity,
                                 bias=ob2_sb[:, 0:1])
            nc.sync.dma_start(dy[:], y2[:])

    nc.compile()
    return nc


def kernel(x, walk_x, params, walk_nodes, batch):
    x = np.asarray(x, dtype=np.float32)
    walk_x = np.asarray(walk_x, dtype=np.float32)
    walk_nodes = np.asarray(walk_nodes, dtype=np.int32)
    batch = np.asarray(batch, dtype=np.int32)
    shared, per_core, T_cell, NT = _host_prep(x, walk_x, params, walk_nodes, batch)
    nc = _build(T_cell, NT)
    in_maps = []
    for c in range(NC_):
        m = dict(shared)
        m.update(per_core[c])
        m = {k: np.ascontiguousarray(v) for k, v in m.items()}
        in_maps.append(m)
    trace = os.environ.get('KERNEL_TRACE', '0') == '1'
    res = run_bass_kernel_spmd(nc, in_maps, core_ids=list(range(NC_)),
                               trace=trace)
    if trace:
        print(f"HW exec time: {res.exec_time_ns} ns")
        if res.instructions_and_trace:
            print("trace:", res.instructions_and_trace[1])
    y = res.results[0]["y"]          # [16, 128]
    return np.ascontiguousarray(y[0:OUT, :].T)   # [128, 10]
